# revision 13
# baseline (speedup 1.0000x reference)
"""Causal self-attention (causal-average variant) Bass kernel for 8 TRN2 cores.

Reference computation (B=4, T=2048, C=1024, fp32):
    v = x @ Wc.T                      # [B,T,C]
    y[b,t,:] = mean_{s<=t} v[b,s,:]   # causal averaging (the per-head split in
                                      # the reference is a no-op: the mask is
                                      # head-independent)
    out = y @ Wp.T                    # [B,T,C]

Algebraic restructuring: there is no nonlinearity between the two
projections, and the causal averaging acts on t while the projections act on
channels, so everything commutes:

    out = diag(1/(t+1)) @ cumsum_t(x) @ (Wp @ Wc).T

The two weight matrices fold into one W = Wp @ Wc on the host (classic
consecutive-linear-layer fusion), the cumsum moves onto x, and the 1/(t+1)
scale lands on the t axis of the output. Per-core device work drops from
three matmul phases (2.7 GMAC) to ONE 1024^3 matmul (1.07 GMAC) plus a DVE
prefix-scan and a fused per-partition scale in the PSUM drain.

Sharding: 8 shards = (batch b in 0..3) x (sequence half j in 0..1), no
collectives. Core gets x[b, 1024j:1024(j+1)].T with the first-half column sum
folded into row 0 for j=1 (cumsum of the local block then equals the global
prefix sum).

Per-core dataflow (all matmul inputs bf16 -> full PE rate; fp32 PSUM):
    DMA   : xT [kt=8,128,1024t] bf16 (2MB), WT=(Wp@Wc).T [kt=8,128,1024d]
            bf16 (2MB), iv [128,8] f32 (1/(t_glob+1) per t-tile column)
    DVE   : xc[kt] = cumsum_t(x[kt])            (8 tensor_tensor_scan's)
    PE    : psum[t128, d1024] = sum_kt xc[kt][:, tt].T @ WT[kt]   (16 MMs/tt)
    ACT   : o[tt] = psum * iv[:, tt]  (activation Copy, per-partition scale
            = the 1/(t+1) averaging denominators; drains PSUM, casts bf16)
    DMA   : o [tt,128,1024] bf16 out (2MB); host assembles + casts fp32.

Engine budget per core/iter: PE 128 matmuls x 512 cols = 65536 cyc @2.4GHz
= 27.3us (the floor for 1.07 GMAC); DVE ~9us; ACT ~8.5us; DMA ~6.2MB ~19us.
Everything but PE hides under the matmul in steady state.
"""
import sys

sys.path.insert(0, "/opt/trn_rl_repo")

import numpy as np
import ml_dtypes

import concourse.bass as bass  # noqa: F401  (import keeps bass registered)
import concourse.tile as tile
from concourse import bacc, mybir
from concourse.bass_utils import run_bass_kernel_spmd

P = 128          # partitions
TH = 1024        # sequence half per core
C = 1024         # channels
NT = TH // P     # 8 t-tiles
NKT = C // P     # 8 k-tiles
ND = 2           # d-halves (512-wide matmul moving blocks)
NB = C // ND     # 512
CORES = list(range(8))

BF = mybir.dt.bfloat16
F32 = mybir.dt.float32

_CACHE = {}


def _build(repeat=1, bench=False, wu=0, x_bufs=2, w_bufs=2, c_bufs=2,
           o_bufs=4, ps_bufs=4, odma="gpsimd", wdma="scalar", sch=1,
           stag=False, hwloop=None):
    nc = bacc.Bacc("TRN2", target_bir_lowering=False, debug=False, num_devices=8)
    # DRAM layouts chosen so every DMA is a contiguous slice. In bench mode
    # the big tensors are Internal (uninitialized garbage — DMA and matmul
    # timing is data-independent) so per-call transfer is tiny.
    kin = "Internal" if bench else "ExternalInput"
    kout = "Internal" if bench else "ExternalOutput"
    x_d = nc.dram_tensor("xt", [NKT, P, TH], BF, kind=kin)   # [kt, p(k), t]
    w_d = nc.dram_tensor("wt", [NKT, P, C], BF, kind=kin)    # [kt, p(k), d]
    iv_d = nc.dram_tensor("iv", [P, NT], F32, kind=kin)      # 1/(t_glob+1)
    o_d = nc.dram_tensor("o", [NT, P, C], BF, kind=kout)     # [tt, p(t), d]
    if bench:
        din_d = nc.dram_tensor("din", [P, 8], F32, kind="ExternalInput")
        dout_d = nc.dram_tensor("dout", [P, 8], F32, kind="ExternalOutput")

    add = mybir.AluOpType.add
    if wu:
        # warmup psum needs its own bank: [P,1024] f32 psum tiles are 2 banks
        # each, so cap at 3 to stay within the 8 PSUM banks.
        ps_bufs = min(ps_bufs, 3)

    with tile.TileContext(nc) as tc:
        with (
            tc.tile_pool(name="x", bufs=1) as x_pool,
            tc.tile_pool(name="w", bufs=1) as w_pool,
            tc.tile_pool(name="c", bufs=1) as c_pool,
            tc.tile_pool(name="o", bufs=1) as o_pool,
            tc.tile_pool(name="m", bufs=1) as m_pool,
            tc.tile_pool(name="ps", bufs=1, space="PSUM") as ps_pool,
        ):

            # Loop-invariant prologue: averaging denominators + the scan's
            # zero operand live outside the repeat loop (no per-iteration
            # re-DMA, and no WAR stall on the FIFO DMA ring at the loop edge).
            iv_t = m_pool.tile([P, NT], F32, tag="iv", name="iv_t", bufs=1)
            nc.sync.dma_start(iv_t[:], iv_d[:])
            zero_t = m_pool.tile([P, TH], BF, tag="z", name="zero_t", bufs=1)
            nc.gpsimd.memset(zero_t[:], 0.0)
            # PE warmup: dummy matmuls with no DMA deps cover the initial
            # HAM clock-gate ramp on the single-shot path.
            if wu:
                wu_t = m_pool.tile([P, NB], BF, tag="wu", name="wu_t", bufs=1)
                nc.gpsimd.memset(wu_t[:], 0.0)
                wu_ps = ps_pool.tile([P, NB], F32, tag="psw", name="wu_ps", bufs=1)
                for _ in range(wu):
                    nc.tensor.matmul(wu_ps[:], wu_t[:, :P], wu_t[:],
                                     start=True, stop=True)

            def body():
                # x DMAs first (scans gate on them) on the SP ring; w DMAs on
                # a second ring so both streams run concurrently and neither
                # head-of-line-blocks the other.
                x_ts, w_ts, c_ts = {}, {}, {}
                for kt in range(NKT):
                    x_ts[kt] = x_pool.tile([P, TH], BF, tag=f"x{kt}",
                                           name=f"x{kt}", bufs=x_bufs)
                    nc.sync.dma_start(x_ts[kt][:], x_d[kt])
                for kt in range(NKT):
                    w_ts[kt] = w_pool.tile([P, C], BF, tag=f"w{kt}",
                                           name=f"w{kt}", bufs=w_bufs)
                    getattr(nc, wdma).dma_start(w_ts[kt][:], w_d[kt])

                # xc[kt] = cumsum over t (free dim) of x[kt]; fp32 scan state,
                # bf16 output feeds the PE as lhsT. For j=1 cores the
                # first-half carry is pre-folded into x row t=0 on the host,
                # so initial=0 still yields the global prefix sum. Chunked
                # along t (state chained via `initial`) so the first matmuls
                # are gated on half-scans, not full ones.
                SC = TH // sch
                for kt in range(NKT):
                    c_ts[kt] = c_pool.tile([P, TH], BF, tag=f"c{kt}",
                                           name=f"c{kt}", bufs=c_bufs)
                for ch in range(sch):
                    sl = slice(ch * SC, (ch + 1) * SC)
                    for kt in range(NKT):
                        nc.vector.tensor_tensor_scan(
                            c_ts[kt][:, sl], x_ts[kt][:, sl], zero_t[:, :SC],
                            0.0 if ch == 0 else c_ts[kt][:, ch * SC - 1:ch * SC],
                            add, add)

                # w[t, d] = sum_k xc[t, k] * W[d, k]; psum [128 t, 1024 d]
                # spans both 512-wide accumulation groups, then one fused
                # drain: o = psum * 1/(t_glob+1) (per-partition scale on the
                # Activation engine, downcast to bf16).
                for tt in range(NT):
                    psum = ps_pool.tile([P, C], F32, tag="ps", bufs=ps_bufs)
                    for dh in range(ND):
                        for kt in range(NKT):
                            nc.tensor.matmul(
                                psum[:, dh * NB:(dh + 1) * NB],
                                c_ts[kt][:, tt * P:(tt + 1) * P],
                                w_ts[kt][:, dh * NB:(dh + 1) * NB],
                                start=(kt == 0), stop=(kt == NKT - 1))
                    o_t = o_pool.tile([P, C], BF, tag="o", bufs=o_bufs)
                    nc.scalar.mul(o_t[:], psum[:], iv_t[:, tt:tt + 1])
                    # Output DMA on a different DGE queue than the input
                    # stream: out-DMAs wait on drains, and FIFO ring order
                    # must not head-of-line-block the next iteration's x/w
                    # loads behind them.
                    getattr(nc, odma).dma_start(o_d[tt], o_t[:])

            if hwloop is None:
                use_loop = bench and repeat > 1
            else:
                use_loop = hwloop and repeat > 1
            if use_loop:
                # For_i ends every iteration with an all-engine barrier +
                # semaphore reset, and a hardware loop re-executes the same
                # baked SBUF addresses — so consecutive loop iterations
                # serialize (no cross-iteration double buffering). Unroll U
                # bodies inside the loop: tag rotation pipelines them, and
                # the barrier cost is amortized 1/U. staggered_reset replaces
                # the back-edge all-engine barrier with a 4-stage rolling
                # reset (stage boundaries between unrolled bodies) so even
                # the chunk edges overlap.
                U = max(u for u in (8, 4, 2, 1) if repeat % u == 0)
                use_stag = stag and U >= 4
                with tc.For_i(0, repeat // U, 1, staggered_reset=use_stag):
                    for _u in range(U):
                        body()
                        if use_stag and _u in (U // 4 - 1, U // 2 - 1,
                                               3 * U // 4 - 1):
                            tc.stage_boundary()
            else:
                for _rep in range(repeat):
                    body()
            if bench:
                with tc.tile_pool(name="dummy", bufs=1) as d_pool:
                    d_t = d_pool.tile([P, 8], F32)
                    nc.sync.dma_start(d_t[:], din_d[:])
                    nc.sync.dma_start(dout_d[:], d_t[:])

    nc.compile()
    return nc


def _get_program(repeat=1, bench=False, **kw):
    if bench:
        kw.setdefault("wu", 0)
    else:
        kw.setdefault("wu", 20)
    key = ("nc", repeat, bench, tuple(sorted(kw.items())))
    if key not in _CACHE:
        _CACHE[key] = _build(repeat, bench, **kw)
    return _CACHE[key]


def _consts():
    # 1/(t_global+1) laid out [p(t), tt] per sequence-half j.
    if "iv" not in _CACHE:
        ivs = []
        for j in range(2):
            tg = (TH * j + np.arange(TH, dtype=np.float32)).reshape(NT, P)
            ivs.append(np.ascontiguousarray((1.0 / (tg + 1.0)).T))  # [p, tt]
        _CACHE["iv"] = ivs
    return _CACHE["iv"]


def _prep_inputs(x, Wc, Wp):
    x = np.ascontiguousarray(np.asarray(x, dtype=np.float32))
    Wc = np.asarray(Wc, dtype=np.float32)
    Wp = np.asarray(Wp, dtype=np.float32)

    # W = Wp @ Wc folds both projections; device consumes W.T = Wc.T @ Wp.T
    # as [p(k), d] tiles.
    wT = np.ascontiguousarray(Wc.T @ Wp.T)                   # [k, d]
    w_in = wT.reshape(NKT, P, C).astype(ml_dtypes.bfloat16)  # [kt, p(k), d]

    ivs = _consts()

    in_maps = []
    for core in CORES:
        b, j = divmod(core, 2)
        xs = x[b, TH * j:TH * (j + 1)].copy()
        if j == 1:
            xs[0] += x[b, :TH].sum(axis=0)
        xt = np.ascontiguousarray(xs.T).reshape(NKT, P, TH)  # [kt, p(k), t]
        in_maps.append({"xt": xt.astype(ml_dtypes.bfloat16),
                        "wt": w_in, "iv": ivs[j]})
    return in_maps


def _run(x, Wc, Wp, trace=False, repeat=1):
    nc = _get_program(repeat)
    in_maps = _prep_inputs(x, Wc, Wp)
    res = run_bass_kernel_spmd(nc, in_maps, CORES, trace=trace)
    B = np.asarray(x).shape[0]
    out = np.empty((B, 2 * TH, C), dtype=np.float32)
    for core in CORES:
        b, j = divmod(core, 2)
        o = res.results[core]["o"]                 # [tt, p(t), d] bf16
        out[b, TH * j:TH * (j + 1)] = o.reshape(TH, C).astype(np.float32)
    return out, res


def kernel(x, Wc, Wp):
    out, _ = _run(x, Wc, Wp, trace=False)
    return out


# revision 14
# speedup vs baseline: 4.5050x; 4.5050x over previous
"""Causal self-attention (causal-average variant) Bass kernel for 8 TRN2 cores.

Reference computation (B=4, T=2048, C=1024, fp32):
    v = x @ Wc.T                      # [B,T,C]
    y[b,t,:] = mean_{s<=t} v[b,s,:]   # causal averaging (the per-head split in
                                      # the reference is a no-op: the mask is
                                      # head-independent)
    out = y @ Wp.T                    # [B,T,C]

Algebraic restructuring: there is no nonlinearity between the two
projections, and the causal averaging acts on t while the projections act on
channels, so everything commutes:

    out = diag(1/(t+1)) @ cumsum_t(x) @ (Wp @ Wc).T

The two weight matrices fold into one W = Wp @ Wc on the host (classic
consecutive-linear-layer fusion), the cumsum moves onto x, and the 1/(t+1)
scale lands on the t axis of the output. Per-core device work drops from
three matmul phases (2.7 GMAC) to ONE 1024^3 matmul (1.07 GMAC) plus a DVE
prefix-scan and a fused per-partition scale in the PSUM drain.

Sharding: 8 shards = (batch b in 0..3) x (sequence half j in 0..1), no
collectives. Core gets x[b, 1024j:1024(j+1)].T with the first-half column sum
folded into row 0 for j=1 (cumsum of the local block then equals the global
prefix sum).

Per-core dataflow (all matmul inputs bf16 -> full PE rate; fp32 PSUM):
    DMA   : xT [kt=8,128,1024t] bf16 (2MB), WT=(Wp@Wc).T [kt=8,128,1024d]
            bf16 (2MB), iv [128,8] f32 (1/(t_glob+1) per t-tile column)
    DVE   : xc[kt] = cumsum_t(x[kt])            (8 tensor_tensor_scan's)
    PE    : psum[t128, d1024] = sum_kt xc[kt][:, tt].T @ WT[kt]   (16 MMs/tt)
    ACT   : o[tt] = psum * iv[:, tt]  (activation Copy, per-partition scale
            = the 1/(t+1) averaging denominators; drains PSUM, casts bf16)
    DMA   : o [tt,128,1024] bf16 out (2MB); host assembles + casts fp32.

Engine budget per core/iter: PE 128 matmuls x 512 cols = 65536 cyc @2.4GHz
= 27.3us (the floor for 1.07 GMAC); DVE ~9us; ACT ~8.5us; DMA ~6.2MB ~19us.
Everything but PE hides under the matmul in steady state.
"""
import sys

sys.path.insert(0, "/opt/trn_rl_repo")

import numpy as np
import ml_dtypes

import concourse.bass as bass  # noqa: F401  (import keeps bass registered)
import concourse.tile as tile
from concourse import bacc, mybir
from concourse.bass_utils import run_bass_kernel_spmd

P = 128          # partitions
TH = 1024        # sequence half per core
C = 1024         # channels
NT = TH // P     # 8 t-tiles
NKT = C // P     # 8 k-tiles
ND = 2           # d-halves (512-wide matmul moving blocks)
NB = C // ND     # 512
CORES = list(range(8))

BF = mybir.dt.bfloat16
F32 = mybir.dt.float32

_CACHE = {}


def _build(repeat=1, bench=False, wu=0, x_bufs=2, w_bufs=2, c_bufs=2,
           o_bufs=4, ps_bufs=4, odma="gpsimd", wdma="scalar", sch=1,
           stag=False, hwloop=None):
    nc = bacc.Bacc("TRN2", target_bir_lowering=False, debug=False, num_devices=8)
    # DRAM layouts chosen so every DMA is a contiguous slice. In bench mode
    # the big tensors are Internal (uninitialized garbage — DMA and matmul
    # timing is data-independent) so per-call transfer is tiny.
    kin = "Internal" if bench else "ExternalInput"
    kout = "Internal" if bench else "ExternalOutput"
    x_d = nc.dram_tensor("xt", [NKT, P, TH], BF, kind=kin)   # [kt, p(k), t]
    w_d = nc.dram_tensor("wt", [NKT, P, C], BF, kind=kin)    # [kt, p(k), d]
    iv_d = nc.dram_tensor("iv", [P, NT], F32, kind=kin)      # 1/(t_glob+1)
    o_d = nc.dram_tensor("o", [NT, P, C], BF, kind=kout)     # [tt, p(t), d]
    if bench:
        din_d = nc.dram_tensor("din", [P, 8], F32, kind="ExternalInput")
        dout_d = nc.dram_tensor("dout", [P, 8], F32, kind="ExternalOutput")

    add = mybir.AluOpType.add
    if wu:
        # warmup psum needs its own bank: [P,1024] f32 psum tiles are 2 banks
        # each, so cap at 3 to stay within the 8 PSUM banks.
        ps_bufs = min(ps_bufs, 3)

    with tile.TileContext(nc) as tc:
        with (
            tc.tile_pool(name="x", bufs=1) as x_pool,
            tc.tile_pool(name="w", bufs=1) as w_pool,
            tc.tile_pool(name="c", bufs=1) as c_pool,
            tc.tile_pool(name="o", bufs=1) as o_pool,
            tc.tile_pool(name="m", bufs=1) as m_pool,
            tc.tile_pool(name="ps", bufs=1, space="PSUM") as ps_pool,
        ):

            # Loop-invariant prologue: averaging denominators + the scan's
            # zero operand live outside the repeat loop (no per-iteration
            # re-DMA, and no WAR stall on the FIFO DMA ring at the loop edge).
            iv_t = m_pool.tile([P, NT], F32, tag="iv", name="iv_t", bufs=1)
            nc.sync.dma_start(iv_t[:], iv_d[:])
            zero_t = m_pool.tile([P, TH], BF, tag="z", name="zero_t", bufs=1)
            nc.gpsimd.memset(zero_t[:], 0.0)
            # PE warmup: dummy matmuls with no DMA deps cover the initial
            # HAM clock-gate ramp on the single-shot path.
            if wu:
                wu_t = m_pool.tile([P, NB], BF, tag="wu", name="wu_t", bufs=1)
                nc.gpsimd.memset(wu_t[:], 0.0)
                wu_ps = ps_pool.tile([P, NB], F32, tag="psw", name="wu_ps", bufs=1)
                for _ in range(wu):
                    nc.tensor.matmul(wu_ps[:], wu_t[:, :P], wu_t[:],
                                     start=True, stop=True)

            def body():
                # x DMAs first (scans gate on them) on the SP ring; w DMAs on
                # a second ring so both streams run concurrently and neither
                # head-of-line-blocks the other.
                x_ts, w_ts, c_ts = {}, {}, {}
                for kt in range(NKT):
                    x_ts[kt] = x_pool.tile([P, TH], BF, tag=f"x{kt}",
                                           name=f"x{kt}", bufs=x_bufs)
                    nc.sync.dma_start(x_ts[kt][:], x_d[kt])
                for kt in range(NKT):
                    w_ts[kt] = w_pool.tile([P, C], BF, tag=f"w{kt}",
                                           name=f"w{kt}", bufs=w_bufs)
                    getattr(nc, wdma).dma_start(w_ts[kt][:], w_d[kt])

                # xc[kt] = cumsum over t (free dim) of x[kt]; fp32 scan state,
                # bf16 output feeds the PE as lhsT. For j=1 cores the
                # first-half carry is pre-folded into x row t=0 on the host,
                # so initial=0 still yields the global prefix sum. Chunked
                # along t (state chained via `initial`) so the first matmuls
                # are gated on half-scans, not full ones.
                SC = TH // sch
                for kt in range(NKT):
                    c_ts[kt] = c_pool.tile([P, TH], BF, tag=f"c{kt}",
                                           name=f"c{kt}", bufs=c_bufs)
                for ch in range(sch):
                    sl = slice(ch * SC, (ch + 1) * SC)
                    for kt in range(NKT):
                        nc.vector.tensor_tensor_scan(
                            c_ts[kt][:, sl], x_ts[kt][:, sl], zero_t[:, :SC],
                            0.0 if ch == 0 else c_ts[kt][:, ch * SC - 1:ch * SC],
                            add, add)

                # w[t, d] = sum_k xc[t, k] * W[d, k]; psum [128 t, 1024 d]
                # spans both 512-wide accumulation groups, then one fused
                # drain: o = psum * 1/(t_glob+1) (per-partition scale on the
                # Activation engine, downcast to bf16).
                for tt in range(NT):
                    psum = ps_pool.tile([P, C], F32, tag="ps", bufs=ps_bufs)
                    for dh in range(ND):
                        for kt in range(NKT):
                            nc.tensor.matmul(
                                psum[:, dh * NB:(dh + 1) * NB],
                                c_ts[kt][:, tt * P:(tt + 1) * P],
                                w_ts[kt][:, dh * NB:(dh + 1) * NB],
                                start=(kt == 0), stop=(kt == NKT - 1))
                    o_t = o_pool.tile([P, C], BF, tag="o", bufs=o_bufs)
                    nc.scalar.mul(o_t[:], psum[:], iv_t[:, tt:tt + 1])
                    # Output DMA on a different DGE queue than the input
                    # stream: out-DMAs wait on drains, and FIFO ring order
                    # must not head-of-line-block the next iteration's x/w
                    # loads behind them.
                    getattr(nc, odma).dma_start(o_d[tt], o_t[:])

            if hwloop is None:
                use_loop = bench and repeat > 1
            else:
                use_loop = hwloop and repeat > 1
            if use_loop:
                # For_i ends every iteration with an all-engine barrier +
                # semaphore reset, and a hardware loop re-executes the same
                # baked SBUF addresses — so consecutive loop iterations
                # serialize (no cross-iteration double buffering). Unroll U
                # bodies inside the loop: tag rotation pipelines them, and
                # the barrier cost is amortized 1/U. staggered_reset replaces
                # the back-edge all-engine barrier with a 4-stage rolling
                # reset (stage boundaries between unrolled bodies) so even
                # the chunk edges overlap.
                # U=8 regresses badly (~155us/iter — loop body exceeds the
                # engines' loop replay capacity); U=4 is the sweet spot.
                U = max(u for u in (4, 2, 1) if repeat % u == 0)
                use_stag = stag and U >= 4
                with tc.For_i(0, repeat // U, 1, staggered_reset=use_stag):
                    for _u in range(U):
                        body()
                        if use_stag and _u in (U // 4 - 1, U // 2 - 1,
                                               3 * U // 4 - 1):
                            tc.stage_boundary()
            else:
                for _rep in range(repeat):
                    body()
            if bench:
                with tc.tile_pool(name="dummy", bufs=1) as d_pool:
                    d_t = d_pool.tile([P, 8], F32)
                    nc.sync.dma_start(d_t[:], din_d[:])
                    nc.sync.dma_start(dout_d[:], d_t[:])

    nc.compile()
    return nc


def _get_program(repeat=1, bench=False, **kw):
    if bench:
        kw.setdefault("wu", 0)
    else:
        kw.setdefault("wu", 20)
    key = ("nc", repeat, bench, tuple(sorted(kw.items())))
    if key not in _CACHE:
        _CACHE[key] = _build(repeat, bench, **kw)
    return _CACHE[key]


def _consts():
    # 1/(t_global+1) laid out [p(t), tt] per sequence-half j.
    if "iv" not in _CACHE:
        ivs = []
        for j in range(2):
            tg = (TH * j + np.arange(TH, dtype=np.float32)).reshape(NT, P)
            ivs.append(np.ascontiguousarray((1.0 / (tg + 1.0)).T))  # [p, tt]
        _CACHE["iv"] = ivs
    return _CACHE["iv"]


def _prep_inputs(x, Wc, Wp):
    x = np.ascontiguousarray(np.asarray(x, dtype=np.float32))
    Wc = np.asarray(Wc, dtype=np.float32)
    Wp = np.asarray(Wp, dtype=np.float32)

    # W = Wp @ Wc folds both projections; device consumes W.T = Wc.T @ Wp.T
    # as [p(k), d] tiles.
    wT = np.ascontiguousarray(Wc.T @ Wp.T)                   # [k, d]
    w_in = wT.reshape(NKT, P, C).astype(ml_dtypes.bfloat16)  # [kt, p(k), d]

    ivs = _consts()

    in_maps = []
    for core in CORES:
        b, j = divmod(core, 2)
        xs = x[b, TH * j:TH * (j + 1)].copy()
        if j == 1:
            xs[0] += x[b, :TH].sum(axis=0)
        xt = np.ascontiguousarray(xs.T).reshape(NKT, P, TH)  # [kt, p(k), t]
        in_maps.append({"xt": xt.astype(ml_dtypes.bfloat16),
                        "wt": w_in, "iv": ivs[j]})
    return in_maps


def _run(x, Wc, Wp, trace=False, repeat=1):
    nc = _get_program(repeat)
    in_maps = _prep_inputs(x, Wc, Wp)
    res = run_bass_kernel_spmd(nc, in_maps, CORES, trace=trace)
    B = np.asarray(x).shape[0]
    out = np.empty((B, 2 * TH, C), dtype=np.float32)
    for core in CORES:
        b, j = divmod(core, 2)
        o = res.results[core]["o"]                 # [tt, p(t), d] bf16
        out[b, TH * j:TH * (j + 1)] = o.reshape(TH, C).astype(np.float32)
    return out, res


def kernel(x, Wc, Wp):
    out, _ = _run(x, Wc, Wp, trace=False)
    return out


# revision 17
# speedup vs baseline: 6.2704x; 1.3919x over previous
"""Causal self-attention (causal-average variant) Bass kernel for 8 TRN2 cores.

Reference computation (B=4, T=2048, C=1024, fp32):
    v = x @ Wc.T                      # [B,T,C]
    y[b,t,:] = mean_{s<=t} v[b,s,:]   # causal averaging (the per-head split in
                                      # the reference is a no-op: the mask is
                                      # head-independent)
    out = y @ Wp.T                    # [B,T,C]

Algebraic restructuring: there is no nonlinearity between the two
projections, and the causal averaging acts on t while the projections act on
channels, so everything commutes:

    out = diag(1/(t+1)) @ cumsum_t(x) @ (Wp @ Wc).T

The two weight matrices fold into one W = Wp @ Wc on the host (classic
consecutive-linear-layer fusion), the cumsum moves onto x, and the 1/(t+1)
scale lands on the t axis of the output. Per-core device work drops from
three matmul phases (2.7 GMAC) to ONE 1024^3 matmul (1.07 GMAC) plus a DVE
prefix-scan and a fused per-partition scale in the PSUM drain.

Sharding: 8 shards = (batch b in 0..3) x (sequence half j in 0..1), no
collectives. Core gets x[b, 1024j:1024(j+1)].T with the first-half column sum
folded into row 0 for j=1 (cumsum of the local block then equals the global
prefix sum).

Per-core dataflow (all matmul inputs bf16 -> full PE rate; fp32 PSUM):
    DMA   : xT [kt=8,128,1024t] bf16 (2MB), WT=(Wp@Wc).T [kt=8,128,1024d]
            bf16 (2MB), iv [128,8] f32 (1/(t_glob+1) per t-tile column)
    DVE   : xc[kt] = cumsum_t(x[kt])            (8 tensor_tensor_scan's)
    PE    : psum[t128, d1024] = sum_kt xc[kt][:, tt].T @ WT[kt]   (16 MMs/tt)
    ACT   : o[tt] = psum * iv[:, tt]  (activation Copy, per-partition scale
            = the 1/(t+1) averaging denominators; drains PSUM, casts bf16)
    DMA   : o [tt,128,1024] bf16 out (2MB); host assembles + casts fp32.

Engine budget per core/iter: PE 128 matmuls x 512 cols = 65536 cyc @2.4GHz
= 27.3us (the floor for 1.07 GMAC); DVE ~9us; ACT ~8.5us; DMA ~6.2MB ~19us.
Everything but PE hides under the matmul in steady state.
"""
import sys

sys.path.insert(0, "/opt/trn_rl_repo")

import numpy as np
import ml_dtypes

import concourse.bass as bass  # noqa: F401  (import keeps bass registered)
import concourse.tile as tile
from concourse import bacc, mybir
from concourse.bass_utils import run_bass_kernel_spmd

P = 128          # partitions
TH = 1024        # sequence half per core
C = 1024         # channels
NT = TH // P     # 8 t-tiles
NKT = C // P     # 8 k-tiles
ND = 2           # d-halves (512-wide matmul moving blocks)
NB = C // ND     # 512
CORES = list(range(8))

BF = mybir.dt.bfloat16
F32 = mybir.dt.float32

_CACHE = {}


def _build(repeat=1, bench=False, wu=0, x_bufs=2, w_bufs=2, c_bufs=2,
           o_bufs=4, ps_bufs=4, odma="gpsimd", wdma="scalar", sch=1,
           stag=False, hwloop=None, pf=True):
    nc = bacc.Bacc("TRN2", target_bir_lowering=False, debug=False, num_devices=8)
    # DRAM layouts chosen so every DMA is a contiguous slice. In bench mode
    # the big tensors are Internal (uninitialized garbage — DMA and matmul
    # timing is data-independent) so per-call transfer is tiny.
    kin = "Internal" if bench else "ExternalInput"
    kout = "Internal" if bench else "ExternalOutput"
    x_d = nc.dram_tensor("xt", [NKT, P, TH], BF, kind=kin)   # [kt, p(k), t]
    w_d = nc.dram_tensor("wt", [NKT, P, C], BF, kind=kin)    # [kt, p(k), d]
    iv_d = nc.dram_tensor("iv", [P, NT], F32, kind=kin)      # 1/(t_glob+1)
    o_d = nc.dram_tensor("o", [NT, P, C], BF, kind=kout)     # [tt, p(t), d]
    if bench:
        din_d = nc.dram_tensor("din", [P, 8], F32, kind="ExternalInput")
        dout_d = nc.dram_tensor("dout", [P, 8], F32, kind="ExternalOutput")

    add = mybir.AluOpType.add
    if wu:
        # warmup psum needs its own bank: [P,1024] f32 psum tiles are 2 banks
        # each, so cap at 3 to stay within the 8 PSUM banks.
        ps_bufs = min(ps_bufs, 3)

    with tile.TileContext(nc) as tc:
        with (
            tc.tile_pool(name="x", bufs=1) as x_pool,
            tc.tile_pool(name="w", bufs=1) as w_pool,
            tc.tile_pool(name="c", bufs=1) as c_pool,
            tc.tile_pool(name="o", bufs=1) as o_pool,
            tc.tile_pool(name="m", bufs=1) as m_pool,
            tc.tile_pool(name="ps", bufs=1, space="PSUM") as ps_pool,
        ):

            # Loop-invariant prologue: averaging denominators + the scan's
            # zero operand live outside the repeat loop (no per-iteration
            # re-DMA, and no WAR stall on the FIFO DMA ring at the loop edge).
            iv_t = m_pool.tile([P, NT], F32, tag="iv", name="iv_t", bufs=1)
            nc.sync.dma_start(iv_t[:], iv_d[:])
            zero_t = m_pool.tile([P, TH], BF, tag="z", name="zero_t", bufs=1)
            nc.gpsimd.memset(zero_t[:], 0.0)
            # PE warmup: dummy matmuls with no DMA deps cover the initial
            # HAM clock-gate ramp on the single-shot path.
            if wu:
                wu_t = m_pool.tile([P, NB], BF, tag="wu", name="wu_t", bufs=1)
                nc.gpsimd.memset(wu_t[:], 0.0)
                wu_ps = ps_pool.tile([P, NB], F32, tag="psw", name="wu_ps", bufs=1)
                for _ in range(wu):
                    nc.tensor.matmul(wu_ps[:], wu_t[:, :P], wu_t[:],
                                     start=True, stop=True)

            def prefetch():
                # Load + scan stage. x DMAs on the SP ring, w on a second
                # ring so both streams run concurrently and neither
                # head-of-line-blocks the other.
                x_ts, w_ts, c_ts = {}, {}, {}
                for kt in range(NKT):
                    x_ts[kt] = x_pool.tile([P, TH], BF, tag=f"x{kt}",
                                           name=f"x{kt}", bufs=x_bufs)
                    nc.sync.dma_start(x_ts[kt][:], x_d[kt])
                for kt in range(NKT):
                    w_ts[kt] = w_pool.tile([P, C], BF, tag=f"w{kt}",
                                           name=f"w{kt}", bufs=w_bufs)
                    getattr(nc, wdma).dma_start(w_ts[kt][:], w_d[kt])

                # xc[kt] = cumsum over t (free dim) of x[kt]; fp32 scan state,
                # bf16 output feeds the PE as lhsT. For j=1 cores the
                # first-half carry is pre-folded into x row t=0 on the host,
                # so initial=0 still yields the global prefix sum. Optionally
                # chunked along t (state chained via `initial`).
                SC = TH // sch
                for kt in range(NKT):
                    c_ts[kt] = c_pool.tile([P, TH], BF, tag=f"c{kt}",
                                           name=f"c{kt}", bufs=c_bufs)
                for ch in range(sch):
                    sl = slice(ch * SC, (ch + 1) * SC)
                    for kt in range(NKT):
                        nc.vector.tensor_tensor_scan(
                            c_ts[kt][:, sl], x_ts[kt][:, sl], zero_t[:, :SC],
                            0.0 if ch == 0 else c_ts[kt][:, ch * SC - 1:ch * SC],
                            add, add)
                return w_ts, c_ts

            def compute(state):
                # w[t, d] = sum_k xc[t, k] * W[d, k]; psum [128 t, 1024 d]
                # spans both 512-wide accumulation groups, then one fused
                # drain: o = psum * 1/(t_glob+1) (per-partition scale on the
                # Activation engine, downcast to bf16).
                w_ts, c_ts = state
                for tt in range(NT):
                    psum = ps_pool.tile([P, C], F32, tag="ps", bufs=ps_bufs)
                    for dh in range(ND):
                        for kt in range(NKT):
                            nc.tensor.matmul(
                                psum[:, dh * NB:(dh + 1) * NB],
                                c_ts[kt][:, tt * P:(tt + 1) * P],
                                w_ts[kt][:, dh * NB:(dh + 1) * NB],
                                start=(kt == 0), stop=(kt == NKT - 1))
                    o_t = o_pool.tile([P, C], BF, tag="o", bufs=o_bufs)
                    nc.scalar.mul(o_t[:], psum[:], iv_t[:, tt:tt + 1])
                    # Output DMA on a third DGE queue: out-DMAs wait on
                    # drains and must not head-of-line-block input loads.
                    getattr(nc, odma).dma_start(o_d[tt], o_t[:])

            # Software pipeline: each body prefetches (DMA + scan) the NEXT
            # body's inputs while computing on the previous prefetch. Across
            # the For_i back edge this means compute starts immediately after
            # the barrier — the last body's prefetch landed pre-barrier (the
            # all-engine barrier orders it), so the loop head costs nothing.
            pstate = {}

            def body():
                if pf:
                    nxt = prefetch()
                    compute(pstate["s"])
                    pstate["s"] = nxt
                else:
                    compute(prefetch())

            if pf:
                pstate["s"] = prefetch()
            if hwloop is None:
                use_loop = bench and repeat > 1
            else:
                use_loop = hwloop and repeat > 1
            if use_loop:
                # For_i ends every iteration with an all-engine barrier +
                # semaphore reset, and a hardware loop re-executes the same
                # baked SBUF addresses — so consecutive loop iterations
                # serialize (no cross-iteration double buffering). Unroll U
                # bodies inside the loop: tag rotation pipelines them, and
                # the barrier cost is amortized 1/U. staggered_reset replaces
                # the back-edge all-engine barrier with a 4-stage rolling
                # reset (stage boundaries between unrolled bodies) so even
                # the chunk edges overlap.
                # U=8 regresses badly (~155us/iter — loop body exceeds the
                # engines' loop replay capacity); U=4 is the sweet spot.
                U = max(u for u in (4, 2, 1) if repeat % u == 0)
                use_stag = stag and U >= 4
                with tc.For_i(0, repeat // U, 1, staggered_reset=use_stag):
                    for _u in range(U):
                        body()
                        if use_stag and _u in (U // 4 - 1, U // 2 - 1,
                                               3 * U // 4 - 1):
                            tc.stage_boundary()
            else:
                for _rep in range(repeat):
                    body()
            if bench:
                with tc.tile_pool(name="dummy", bufs=1) as d_pool:
                    d_t = d_pool.tile([P, 8], F32)
                    nc.sync.dma_start(d_t[:], din_d[:])
                    nc.sync.dma_start(dout_d[:], d_t[:])

    nc.compile()
    return nc


def _get_program(repeat=1, bench=False, **kw):
    if bench:
        kw.setdefault("wu", 0)
    else:
        kw.setdefault("wu", 20)
    key = ("nc", repeat, bench, tuple(sorted(kw.items())))
    if key not in _CACHE:
        _CACHE[key] = _build(repeat, bench, **kw)
    return _CACHE[key]


def _consts():
    # 1/(t_global+1) laid out [p(t), tt] per sequence-half j.
    if "iv" not in _CACHE:
        ivs = []
        for j in range(2):
            tg = (TH * j + np.arange(TH, dtype=np.float32)).reshape(NT, P)
            ivs.append(np.ascontiguousarray((1.0 / (tg + 1.0)).T))  # [p, tt]
        _CACHE["iv"] = ivs
    return _CACHE["iv"]


def _prep_inputs(x, Wc, Wp):
    x = np.ascontiguousarray(np.asarray(x, dtype=np.float32))
    Wc = np.asarray(Wc, dtype=np.float32)
    Wp = np.asarray(Wp, dtype=np.float32)

    # W = Wp @ Wc folds both projections; device consumes W.T = Wc.T @ Wp.T
    # as [p(k), d] tiles.
    wT = np.ascontiguousarray(Wc.T @ Wp.T)                   # [k, d]
    w_in = wT.reshape(NKT, P, C).astype(ml_dtypes.bfloat16)  # [kt, p(k), d]

    ivs = _consts()

    in_maps = []
    for core in CORES:
        b, j = divmod(core, 2)
        xs = x[b, TH * j:TH * (j + 1)].copy()
        if j == 1:
            xs[0] += x[b, :TH].sum(axis=0)
        xt = np.ascontiguousarray(xs.T).reshape(NKT, P, TH)  # [kt, p(k), t]
        in_maps.append({"xt": xt.astype(ml_dtypes.bfloat16),
                        "wt": w_in, "iv": ivs[j]})
    return in_maps


def _run(x, Wc, Wp, trace=False, repeat=1):
    nc = _get_program(repeat)
    in_maps = _prep_inputs(x, Wc, Wp)
    res = run_bass_kernel_spmd(nc, in_maps, CORES, trace=trace)
    B = np.asarray(x).shape[0]
    out = np.empty((B, 2 * TH, C), dtype=np.float32)
    for core in CORES:
        b, j = divmod(core, 2)
        o = res.results[core]["o"]                 # [tt, p(t), d] bf16
        out[b, TH * j:TH * (j + 1)] = o.reshape(TH, C).astype(np.float32)
    return out, res


def kernel(x, Wc, Wp):
    out, _ = _run(x, Wc, Wp, trace=False)
    return out
